# revision 1
# baseline (speedup 1.0000x reference)
"""Trainium2 Bass kernel for CustomPositionsPiecewiseConv2d.

Math: for knots positions=[-1,-.5,0,.5,1] and inputs x in [0,1], the active
interpolation coefficients are
    c2 = relu(1-2v),  c4 = max(relu(2v-1), T),  c3 = 1 - c2 - c4
with T = 1[v >= theta] the isclose(v,1) mask.  Since c2+c3+c4 == 1 exactly
(everywhere, including the zero-padding border), the c3 plane folds away:
    out = C2 (x) (W2-W3) + C4 (x) (W4-W3) + sum_ck W3[o,c,k] + bias
Each plane is an elementwise function of v and v is shifted/padded copies of x,
so planes are computed once per padded image and the 3x3 im2col becomes 9
shifted access-pattern reads feeding PSUM-accumulated matmuls.

Modes:
  float32       exact, 4 cyc/row on PE
  float32r      tf32-rounded operands, 1 cyc/row (err ~1.6e-4)
  float32r_split  hi/lo tf32 split of coeffs+weights; per tap one K=128 matmul
                  [c2h,c4h,c2l,c4l]x[W2h,W4h,W2h,W4h] plus one K=64 matmul
                  [c2h,c4h]x[W2l,W4l]; drops only (lo x lo) terms ~2^-24.

Sharding: data-parallel over batch, 2 images per core on 8 cores.
"""

import numpy as np

B, C, H, W = 16, 32, 64, 64
O, P, KH, KW = 128, 5, 3, 3
NCORES = 8
IPC = B // NCORES            # images per core
HP, WP = H + 2, W + 2        # padded image (pad=1)
RT = 8                       # output rows per L-tile
NT = H // RT                 # L-tiles per image
K2 = KH * KW
L = H * W
ATOL = 1e-5
RTOL = 1e-5

MODE = "bf16_split"          # float32 | float32r | bf16_split


# ---------------------------------------------------------------- host math


def _isclose_np(a, b):
    return np.abs(a - b) <= np.float32(ATOL) + np.float32(RTOL) * np.abs(b)


def _reference_np(x, weights, bias, positions):
    """Direct numpy port of the reference (fallback path)."""
    EPS = 1e-6
    Bn, Cn, Hn, Wn = x.shape
    On, _, Pn, KHn, KWn = weights.shape
    xp = np.pad(x, ((0, 0), (0, 0), (1, 1), (1, 1)))
    cols = [
        xp[:, :, i : i + Hn, j : j + Wn] for i in range(KHn) for j in range(KWn)
    ]
    pat = np.stack(cols, axis=2)
    v = pat.reshape(Bn, Cn, KHn * KWn, Hn * Wn).astype(np.float32)

    left, right = positions[:-1], positions[1:]
    denom = right - left
    denom = np.where(denom == 0, np.float32(EPS), denom)
    varc = (1.0 / denom).astype(np.float32)
    const = (-left * varc).astype(np.float32)

    m_first = _isclose_np(v, positions[0])
    m_last = _isclose_np(v, positions[-1])
    in_range = (~(m_first | m_last)) & (v >= positions[0]) & (v <= positions[-1])

    coeff = np.zeros(v.shape + (Pn,), np.float32)
    coeff[..., 0] += m_first.astype(np.float32)
    coeff[..., Pn - 1] += m_last.astype(np.float32)
    for p in range(Pn - 1):
        m = (in_range & (v >= positions[p]) & (v < positions[p + 1])).astype(
            np.float32
        )
        t = v * varc[p] + const[p]
        coeff[..., p] += m * (1.0 - t)
        coeff[..., p + 1] += m * t

    Wk = np.transpose(weights, (0, 1, 3, 4, 2)).reshape(On, Cn, KHn * KWn, Pn)
    ident = np.all(np.abs(Wk - 1.0) <= np.float32(ATOL + RTOL), axis=-1)
    Wk_eff = np.where(ident[..., None], np.float32(0.0), Wk)

    out = np.einsum("bcklp,ockp->bol", coeff, Wk_eff, optimize=True)
    out = out + np.einsum(
        "bckl,ock->bol", v, ident.astype(np.float32), optimize=True
    )
    out = out + bias[None, :, None]
    return out.reshape(Bn, On, Hn, Wn).astype(np.float32)


def _compute_theta():
    """Smallest fp32 v such that fp32(1-v) <= fp32(ATOL + RTOL*1.0), matching
    the reference's m_last = isclose(v, 1.0) for v <= 1."""
    tau = np.float32(np.float32(ATOL) + np.float32(RTOL) * np.float32(1.0))
    th = np.float32(np.float32(1.0) - tau)
    while np.float32(np.float32(1.0) - np.nextafter(th, np.float32(0.0))) <= tau:
        th = np.nextafter(th, np.float32(0.0))
    while np.float32(np.float32(1.0) - th) > tau:
        th = np.nextafter(th, np.float32(2.0))
    return np.float32(th)


def _host_weights(weights, bias):
    """Fold c3 away.  Returns (wfold [2C, K2, O] f32 = [W2-W3; W4-W3],
    bias_eff [O] f32 = bias + sum_ck W3, ident_any)."""
    Wk = np.transpose(weights, (0, 1, 3, 4, 2)).reshape(O, C, K2, P)
    ident = np.all(np.abs(Wk - 1.0) <= np.float32(ATOL + RTOL), axis=-1)
    ident_any = bool(ident.any())
    Wk_eff = np.where(ident[..., None], np.float32(0.0), Wk)
    W2 = Wk_eff[:, :, :, 2].astype(np.float64)
    W3 = Wk_eff[:, :, :, 3].astype(np.float64)
    W4 = Wk_eff[:, :, :, 4].astype(np.float64)
    wfold = np.zeros((2 * C, K2, O), np.float32)
    wfold[0:C] = (W2 - W3).astype(np.float32).transpose(1, 2, 0)
    wfold[C : 2 * C] = (W4 - W3).astype(np.float32).transpose(1, 2, 0)
    bias_eff = (bias.astype(np.float64) + W3.sum(axis=(1, 2))).astype(np.float32)
    return np.ascontiguousarray(wfold), np.ascontiguousarray(bias_eff), ident_any


# ---------------------------------------------------------------- device IR


def _build_nc(theta, mode):
    import concourse.tile as tile
    from concourse import bacc, mybir

    f32 = mybir.dt.float32
    f32r = mybir.dt.float32r
    bf16 = mybir.dt.bfloat16
    Alu = mybir.AluOpType
    Act = mybir.ActivationFunctionType
    split = mode == "bf16_split"
    if mode == "float32":
        plane_dt = f32
    elif mode == "float32r":
        plane_dt = f32r
    else:
        plane_dt = bf16

    nc = bacc.Bacc("TRN2", target_bir_lowering=False, debug=False,
                   num_devices=NCORES)
    x_d = nc.dram_tensor("x", [IPC, C, H, W], f32, kind="ExternalInput").ap()
    w_d = nc.dram_tensor("wfold", [2 * C, K2, O], f32, kind="ExternalInput").ap()
    b_d = nc.dram_tensor("bias", [O, 1], f32, kind="ExternalInput").ap()
    o_d = nc.dram_tensor("out", [IPC, O, H, W], f32, kind="ExternalOutput").ap()

    with tile.TileContext(nc) as tc:
        with (
            tc.tile_pool(name="const", bufs=1) as constp,
            tc.tile_pool(name="scratch", bufs=1) as scrp,
            tc.tile_pool(name="plane", bufs=1) as planep,
            tc.tile_pool(name="ybuf", bufs=2) as ybufp,
            tc.tile_pool(name="psum", bufs=1, space="PSUM") as psump,
            tc.tile_pool(name="osb", bufs=4) as osbp,
        ):
            # ---- x loads first (phi critical path), weights after ----
            XF = scrp.tile([IPC * C, H, W], f32)      # flat x
            for i in range(IPC):
                nc.sync.dma_start(XF[i * C : (i + 1) * C], x_d[i])

            # pull the ACT table load off the critical path
            tiny = constp.tile([C, 1], f32)
            nc.gpsimd.memset(tiny[:], 0.0)
            nc.scalar.activation(tiny[:], tiny[:], Act.Relu, bias=0.0, scale=1.0)

            # PE warmup: dummy matmuls keep HAM at K=8/8 until the real
            # stream starts (otherwise the first ~5us of matmuls run at 1.2GHz)
            zb = constp.tile([128, 512], plane_dt)
            nc.gpsimd.memset(
                zb[:].bitcast(f32) if plane_dt == f32r else zb[:], 0.0
            )
            warm_ctr = [0]

            def warm(nmm, rhs=None):
                """Dummy matmuls (results never read). rhs gates when the
                batch can start, chaining PE busy-ness across the phi phase."""
                w = warm_ctr[0]
                warm_ctr[0] += 1
                pw = psump.tile(
                    [O, 512], f32, name=f"ps_warm{w}", tag=f"ps{w % 2}"
                )
                r = zb[:] if rhs is None else rhs
                kp = r.shape[0]
                for j in range(nmm):
                    nc.tensor.matmul(
                        pw[:], zb[0:kp, 0:128], r,
                        start=(j % 8 == 0), stop=(j % 8 == 7 or j == nmm - 1),
                    )

            warm(16)

            # ---- weights ----
            w_sb = constp.tile([2 * C, K2, O], f32)
            nc.sync.dma_start(w_sb[:], w_d[:])
            b_sb = constp.tile([O, 1], f32)
            nc.sync.dma_start(b_sb[:], b_d[:])
            if mode == "float32":
                w_hi = w_sb
            else:
                w_hi = constp.tile([2 * C, K2, O], plane_dt)
                nc.vector.tensor_copy(w_hi[:], w_sb[:])
            if split:
                w_lo = constp.tile([2 * C, K2, O], plane_dt)
                nc.vector.tensor_tensor(w_lo[:], w_sb[:], w_hi[:], Alu.subtract)
                # lhsT1 rows: [W2h, W4h, W2h, W4h] (hi coeffs then lo coeffs)
                w_rep = constp.tile([4 * C, K2, O], plane_dt)
                nc.sync.dma_start(w_rep[0 : 2 * C], w_hi[:])
                nc.sync.dma_start(w_rep[2 * C : 4 * C], w_hi[:])
                lhs1, lhs2 = w_rep, w_lo
            else:
                lhs1, lhs2 = w_hi, None

            # ---- coefficient planes ----
            # scratch on the same partitions as each image's plane slice
            # (engine ops require equal SBUF base partitions across operands)
            RF = scrp.tile([IPC * C, H, W], f32)
            CF = scrp.tile([IPC * C, H, W], f32)

            npl = 4 if split else 2
            # plane buffers, padded layout; group order:
            #   split: [c2h, c4h, c2l, c4l]   else: [c2, c4]
            PL = [
                planep.tile([IPC * C, HP, WP], plane_dt, name=f"PL{g}")
                for g in range(npl)
            ]
            # borders: c2-like planes = 1 at v=0, everything else = 0
            # (memset rejects f32r dests; same-size bitcast to f32 is a no-op)
            for g, pl in enumerate(PL):
                bv = 1.0 if g == 0 else 0.0
                for strip in (
                    pl[:, 0, :],
                    pl[:, HP - 1, :],
                    pl[:, 1 : HP - 1, 0],
                    pl[:, 1 : HP - 1, WP - 1],
                ):
                    nc.gpsimd.memset(
                        strip.bitcast(f32) if plane_dt == f32r else strip, bv
                    )

            def interior(pl):
                return pl[:, 1 : HP - 1, 1 : WP - 1]

            negone = constp.tile([IPC * C, 1], f32)
            nc.gpsimd.memset(negone[:], -1.0)

            def phi_chunk(r0, r1):
                """Coefficient planes for image rows [r0, r1), both images at
                once (64 partitions). Chunking lets the first GEMM tiles start
                while the rest of the planes are still being computed."""
                xf = XF[:, r0:r1, :]
                neg = negone[:]
                rf = RF[:, r0:r1, :]
                cf = CF[:, r0:r1, :]
                pls = [pl[:, 1 + r0 : 1 + r1, 1 : WP - 1] for pl in PL]
                if split:
                    # bf16 rounding absorbs the isclose(v,1) mask: for
                    # v >= 1-2^-9, relu(2v-1) rounds to exactly 1.0, and the
                    # lo-plane residual lands on hi weights scaled 2^-9.
                    c2h, c4h, c2l, c4l = pls
                    nc.scalar.activation(rf, xf, Act.Relu, bias=neg, scale=2.0)
                    nc.vector.tensor_copy(c4h, rf)
                    nc.vector.tensor_tensor(c4l, rf, c4h, Alu.subtract)
                    nc.scalar.activation(cf, xf, Act.Relu, bias=1.0, scale=-2.0)
                    nc.scalar.activation(c2h, cf, Act.Copy)
                    nc.vector.tensor_tensor(c2l, cf, c2h, Alu.subtract)
                else:
                    c2, c4 = pls
                    nc.vector.tensor_scalar(cf, xf, float(theta), None, Alu.is_ge)
                    nc.scalar.activation(rf, xf, Act.Relu, bias=neg, scale=2.0)
                    nc.vector.tensor_tensor(rf, rf, cf, Alu.max)
                    nc.vector.tensor_copy(c4, rf)
                    nc.scalar.activation(c2, xf, Act.Relu, bias=1.0, scale=-2.0)

            phi_chunk(0, H)
            # bridge PE busy-ness across the phi phase: each batch is gated
            # on a progressively later plane artifact (HAM re-throttles after
            # ~3.4us of PE idle, and a cold stream runs at 1.2GHz)
            nh = npl * C // 2
            order = [1, 3, 0, 2] if split else [1, 0]
            for g in order:
                warm(8, PL[g][0:nh, 0:RT, 0:W])

            # ---- per-image gather + GEMM ----
            # Tap-outer loop: one LDWEIGHTS feeds a run of back-to-back
            # matmuls (same stationary operand), so drain overlaps the next
            # fill and the per-MM cost stays ~N/2.4 instead of the isolated
            # latency. All 8 L-tiles of an image accumulate in 8 PSUM banks.
            # Tiles are issued in two groups gated on the two phi row-chunks
            # (Tile deps are byte-range granular), so the GEMM starts as soon
            # as the first chunk of planes is gathered into Y.
            def tap_loop(Y, pss, tiles, start, stop):
                for ki in range(K2):
                    kh, kw = divmod(ki, KW)
                    cols = slice(kw, kw + W)
                    last = ki == K2 - 1
                    for t in tiles:
                        rows = slice(t * RT + kh, t * RT + kh + RT)
                        nc.tensor.matmul(
                            pss[t][:], lhs1[:, ki, :], Y[:, rows, cols],
                            start=(start and ki == 0),
                            stop=(stop and last and not split),
                        )
                    if split:
                        for t in tiles:
                            rows = slice(t * RT + kh, t * RT + kh + RT)
                            nc.tensor.matmul(
                                pss[t][:], lhs2[:, ki, :],
                                Y[0 : 2 * C, rows, cols],
                                start=False, stop=(stop and last),
                            )

            for i in range(IPC):
                Y = ybufp.tile([npl * C, HP, WP], plane_dt, name="Y", tag="Y")
                s = slice(i * C, (i + 1) * C)
                for g, pl in enumerate(PL):
                    nc.sync.dma_start(Y[g * C : (g + 1) * C], pl[s])

                pss = [
                    psump.tile([O, RT * W], f32, name=f"ps{t}", tag=f"ps{t}")
                    for t in range(NT)
                ]
                tap_loop(Y, pss, list(range(NT)), start=True, stop=True)
                for t in range(NT):
                    osb = osbp.tile([O, RT * W], f32, name="osb")
                    if t % 2 == 0:
                        nc.scalar.activation(
                            osb[:], pss[t][:], Act.Identity, bias=b_sb[:, 0:1],
                            scale=1.0,
                        )
                    else:
                        nc.vector.tensor_scalar(
                            osb[:], pss[t][:], b_sb[:, 0:1], None, Alu.add
                        )
                    nc.sync.dma_start(
                        o_d[i, :, t * RT : (t + 1) * RT, :],
                        osb[:].rearrange("o (r w) -> o r w", r=RT),
                    )
    nc.compile()
    return nc


# ---------------------------------------------------------------- entry


def _prep(inputs):
    x = np.ascontiguousarray(np.asarray(inputs["x"], dtype=np.float32))
    weights = np.ascontiguousarray(np.asarray(inputs["weights"], dtype=np.float32))
    bias = np.ascontiguousarray(np.asarray(inputs["bias"], dtype=np.float32))
    positions = np.ascontiguousarray(
        np.asarray(inputs["positions"], dtype=np.float32)
    )
    return x, weights, bias, positions


def _fast_path_ok(x, positions):
    expect = np.linspace(-1.0, 1.0, P, dtype=np.float32)
    return (
        x.shape == (B, C, H, W)
        and positions.shape == (P,)
        and np.array_equal(positions, expect)
        and float(x.min()) >= 0.0
        and float(x.max()) <= 1.0
    )


def kernel(**inputs):
    x, weights, bias, positions = _prep(inputs)
    if not _fast_path_ok(x, positions):
        return _reference_np(x, weights, bias, positions)

    wfold, bias_eff, ident_any = _host_weights(weights, bias)
    if ident_any:
        # identity-shortcut weights present: needs the raw-v plane; use the
        # exact fallback rather than a rarely-exercised device path
        return _reference_np(x, weights, bias, positions)

    from concourse.bass_utils import run_bass_kernel_spmd

    nc = _build_nc(_compute_theta(), MODE)
    bias2d = np.ascontiguousarray(bias_eff.reshape(O, 1))
    in_maps = [
        {"x": np.ascontiguousarray(x[i * IPC : (i + 1) * IPC]),
         "wfold": wfold, "bias": bias2d}
        for i in range(NCORES)
    ]
    res = run_bass_kernel_spmd(nc, in_maps, core_ids=list(range(NCORES)))
    out = np.concatenate([res.results[i]["out"] for i in range(NCORES)], axis=0)
    return np.ascontiguousarray(out)


# ------------------------------------------------------------ dev utilities


def _run_sim(inputs):
    """CoreSim single-core run (images 0..IPC-1) for correctness debugging."""
    from concourse.bass_interp import CoreSim

    x, weights, bias, positions = _prep(inputs)
    assert _fast_path_ok(x, positions)
    wfold, bias_eff, ident_any = _host_weights(weights, bias)
    assert not ident_any
    nc = _build_nc(_compute_theta(), MODE)
    sim = CoreSim(nc)
    sim.tensor("x")[:] = x[:IPC]
    sim.tensor("wfold")[:] = wfold
    sim.tensor("bias")[:] = bias_eff.reshape(O, 1)
    sim.simulate()
    return np.array(sim.tensor("out"))



# revision 6
# speedup vs baseline: 2.1956x; 2.1956x over previous
"""Trainium2 Bass kernel for CustomPositionsPiecewiseConv2d.

Math: for knots positions=[-1,-.5,0,.5,1] and x in [0,1], only the last two
spline intervals are active.  With g2 = 2*min(v,0.5) and c4 = relu(2v-1)
(bf16 rounding absorbs the isclose(v,1) mask), the per-tap contribution is
    contrib = W3 + (1-g2)*(W2-W3) + c4*(W4-W3)
            = W2 + g2*(W3-W2) + c4*(W4-W3)
so  out = G2 (x) (W3-W2) + C4 (x) (W4-W3) + bias + sum_ck W2.

GEMM packing: contraction over (2 planes x 32 ch x 9 taps) = 576 lanes.
Per image one Y buffer [128, 66, 66] holds [g2; c4; g2 row+1; c4 row+1];
a K=128 matmul then contracts taps (0,kw) and (1,kw) simultaneously
(kw = read-time column offset), and the kh=2 row uses K=64 matmuls on the
lower half: 6 matmuls per output tile instead of 9 (or 18 split).  All
plane->Y copies are full-width row windows (contiguous DMA descriptors).

Output is written [O, IPC, H, W] (per-partition-contiguous 4KB segments)
and transposed on host.  Sharding: data-parallel, 2 images per core.
"""

import numpy as np

B, C, H, W = 16, 32, 64, 64
O, P, KH, KW = 128, 5, 3, 3
NCORES = 8
IPC = B // NCORES            # images per core
HP, WP = H + 2, W + 2        # padded plane (pad=1)
RT = 8                       # output rows per L-tile
NT = H // RT                 # L-tiles per image
K2 = KH * KW
NS = 6                       # stationary steps: 3 pair (K=128) + 3 single (K=64)
ATOL = 1e-5
RTOL = 1e-5

# x row chunks for the load->phi->gather->matmul pipeline
RC = [(0, 33), (33, 64)]


# ---------------------------------------------------------------- host math


def _isclose_np(a, b):
    return np.abs(a - b) <= np.float32(ATOL) + np.float32(RTOL) * np.abs(b)


def _reference_np(x, weights, bias, positions):
    """Direct numpy port of the reference (fallback path)."""
    EPS = 1e-6
    Bn, Cn, Hn, Wn = x.shape
    On, _, Pn, KHn, KWn = weights.shape
    xp = np.pad(x, ((0, 0), (0, 0), (1, 1), (1, 1)))
    cols = [
        xp[:, :, i : i + Hn, j : j + Wn] for i in range(KHn) for j in range(KWn)
    ]
    pat = np.stack(cols, axis=2)
    v = pat.reshape(Bn, Cn, KHn * KWn, Hn * Wn).astype(np.float32)

    left, right = positions[:-1], positions[1:]
    denom = right - left
    denom = np.where(denom == 0, np.float32(EPS), denom)
    varc = (1.0 / denom).astype(np.float32)
    const = (-left * varc).astype(np.float32)

    m_first = _isclose_np(v, positions[0])
    m_last = _isclose_np(v, positions[-1])
    in_range = (~(m_first | m_last)) & (v >= positions[0]) & (v <= positions[-1])

    coeff = np.zeros(v.shape + (Pn,), np.float32)
    coeff[..., 0] += m_first.astype(np.float32)
    coeff[..., Pn - 1] += m_last.astype(np.float32)
    for p in range(Pn - 1):
        m = (in_range & (v >= positions[p]) & (v < positions[p + 1])).astype(
            np.float32
        )
        t = v * varc[p] + const[p]
        coeff[..., p] += m * (1.0 - t)
        coeff[..., p + 1] += m * t

    Wk = np.transpose(weights, (0, 1, 3, 4, 2)).reshape(On, Cn, KHn * KWn, Pn)
    ident = np.all(np.abs(Wk - 1.0) <= np.float32(ATOL + RTOL), axis=-1)
    Wk_eff = np.where(ident[..., None], np.float32(0.0), Wk)

    out = np.einsum("bcklp,ockp->bol", coeff, Wk_eff, optimize=True)
    out = out + np.einsum(
        "bckl,ock->bol", v, ident.astype(np.float32), optimize=True
    )
    out = out + bias[None, :, None]
    return out.reshape(Bn, On, Hn, Wn).astype(np.float32)


def _host_weights(weights, bias):
    """Fold the spline into two folded-weight blocks per tap.

    Returns (wstat [128, NS, O] f32, bias_eff [O] f32, ident_any).
    Stationary step s<3 (kw=s): rows = [-P(0,kw); Q(0,kw); -P(1,kw); Q(1,kw)]
    by 32-channel blocks; step s>=3 (kw=s-3): rows = [-P(2,kw); Q(2,kw); 0; 0]
    where P = W2-W3, Q = W4-W3 (so -P multiplies g2 and Q multiplies c4).
    """
    Wk = np.transpose(weights, (0, 1, 3, 4, 2)).reshape(O, C, K2, P)
    ident = np.all(np.abs(Wk - 1.0) <= np.float32(ATOL + RTOL), axis=-1)
    ident_any = bool(ident.any())
    W2 = Wk[:, :, :, 2].astype(np.float64)
    W3 = Wk[:, :, :, 3].astype(np.float64)
    W4 = Wk[:, :, :, 4].astype(np.float64)
    Pm = W2 - W3                      # [O, C, K2]
    Qm = W4 - W3
    wstat = np.zeros((128, NS, O), np.float64)
    for kw in range(KW):
        wstat[0:32, kw] = -Pm[:, :, 0 * KW + kw].T
        wstat[32:64, kw] = Qm[:, :, 0 * KW + kw].T
        wstat[64:96, kw] = -Pm[:, :, 1 * KW + kw].T
        wstat[96:128, kw] = Qm[:, :, 1 * KW + kw].T
        wstat[0:32, KW + kw] = -Pm[:, :, 2 * KW + kw].T
        wstat[32:64, KW + kw] = Qm[:, :, 2 * KW + kw].T
    bias_eff = (bias.astype(np.float64) + W2.sum(axis=(1, 2))).astype(np.float32)
    return (
        np.ascontiguousarray(wstat.astype(np.float32)),
        bias_eff,
        ident_any,
    )


# ---------------------------------------------------------------- device IR


def _build_nc():
    import concourse.tile as tile
    from concourse import bacc, mybir

    f32 = mybir.dt.float32
    bf16 = mybir.dt.bfloat16
    Alu = mybir.AluOpType
    Act = mybir.ActivationFunctionType

    nc = bacc.Bacc("TRN2", target_bir_lowering=False, debug=False,
                   num_devices=NCORES)
    x_d = nc.dram_tensor("x", [IPC, C, H, W], f32, kind="ExternalInput").ap()
    w_d = nc.dram_tensor("wstat", [128, NS, O], f32, kind="ExternalInput").ap()
    b_d = nc.dram_tensor("bias", [O, 1], f32, kind="ExternalInput").ap()
    o_d = nc.dram_tensor("out", [O, IPC, H, W], f32, kind="ExternalOutput").ap()

    with tile.TileContext(nc) as tc:
        with (
            tc.tile_pool(name="const", bufs=1) as constp,
            tc.tile_pool(name="plane", bufs=1) as planep,
            tc.tile_pool(name="ybuf", bufs=2) as ybufp,
            tc.tile_pool(name="psum", bufs=1, space="PSUM") as psump,
            tc.tile_pool(name="osb", bufs=3) as osbp,
        ):
            # ---- x loads first (phi critical path) ----
            XF = planep.tile([IPC * C, H, W], f32)
            for r0, r1 in RC:
                nc.sync.dma_start(
                    XF[:, r0:r1, :],
                    x_d[:, :, r0:r1, :].rearrange("i c h w -> (i c) h w"),
                )

            # pull the ACT table load off the critical path
            tiny = constp.tile([C, 1], f32)
            nc.gpsimd.memset(tiny[:], 0.0)
            nc.scalar.activation(tiny[:], tiny[:], Act.Relu, bias=0.0, scale=1.0)

            # PE warmup: dummy matmuls keep HAM at 8/8 until the real stream
            zb = constp.tile([128, 512], bf16)
            nc.gpsimd.memset(zb[:], 0.0)
            warm_ctr = [0]

            def warm(nmm, rhs=None):
                w = warm_ctr[0]
                warm_ctr[0] += 1
                pw = psump.tile(
                    [O, 512], f32, name=f"ps_warm{w}", tag=f"ps{w % 2}"
                )
                r = zb[:] if rhs is None else rhs
                kp = r.shape[0]
                for j in range(nmm):
                    nc.tensor.matmul(
                        pw[:], zb[0:kp, 0:128], r,
                        start=(j % 8 == 0), stop=(j % 8 == 7 or j == nmm - 1),
                    )

            warm(12)

            # ---- weights ----
            w_sb = constp.tile([128, NS, O], f32)
            nc.sync.dma_start(w_sb[:], w_d[:])
            b_sb = constp.tile([O, 1], f32)
            nc.sync.dma_start(b_sb[:], b_d[:])

            # ---- coefficient planes (both images, 64 partitions) ----
            TG2 = planep.tile([IPC * C, HP, WP], bf16)
            TC4 = planep.tile([IPC * C, HP, WP], bf16)
            for T in (TG2, TC4):
                for strip in (
                    T[:, 0, :],
                    T[:, HP - 1, :],
                    T[:, 1 : HP - 1, 0],
                    T[:, 1 : HP - 1, WP - 1],
                ):
                    nc.gpsimd.memset(strip, 0.0)
            negone = constp.tile([IPC * C, 1], f32)
            nc.gpsimd.memset(negone[:], -1.0)
            for r0, r1 in RC:
                nc.vector.tensor_scalar(
                    TG2[:, 1 + r0 : 1 + r1, 1 : W + 1], XF[:, r0:r1, :],
                    0.5, 2.0, Alu.min, Alu.mult,
                )
                nc.scalar.activation(
                    TC4[:, 1 + r0 : 1 + r1, 1 : W + 1], XF[:, r0:r1, :],
                    Act.Relu, bias=negone[:], scale=2.0,
                )

            w16 = constp.tile([128, NS, O], bf16)
            nc.vector.tensor_copy(w16[:], w_sb[:])

            # bridge PE busy-ness across the phi phase (HAM re-throttles
            # after ~3.4us idle); each batch gated on a later phi artifact
            warm(6, TG2[0:64, 0:RT, 0:W])
            warm(6, TC4[0:64, 0:RT, 0:W])

            # ---- per-image gather + GEMM ----
            def gather(i, Y, chunks):
                # chunks: disjoint ((lo0, lo1), (hi0, hi1)) plane-row ranges;
                # lower 64 parts hold plane rows r, upper 64 hold rows r+1
                s = slice(i * C, (i + 1) * C)
                for (lo0, lo1), (hi0, hi1) in chunks:
                    nc.sync.dma_start(Y[0:32, lo0:lo1, :], TG2[s, lo0:lo1, :])
                    nc.sync.dma_start(Y[32:64, lo0:lo1, :], TC4[s, lo0:lo1, :])
                    nc.sync.dma_start(
                        Y[64:96, hi0:hi1, :], TG2[s, hi0 + 1 : hi1 + 1, :]
                    )
                    nc.sync.dma_start(
                        Y[96:128, hi0:hi1, :], TC4[s, hi0 + 1 : hi1 + 1, :]
                    )

            for i in range(IPC):
                Y = ybufp.tile([128, HP, WP], bf16, name="Y", tag="Y")
                if i == 0:
                    # chunked: matmul tiles 0-3 start after the first chunk
                    gather(i, Y, [((0, 34), (0, 33)), ((34, HP), (33, H))])
                else:
                    gather(i, Y, [((0, HP), (0, H))])

                pss = [
                    psump.tile([O, RT * W], f32, name=f"ps{t}", tag=f"ps{t}")
                    for t in range(NT)
                ]
                # stationary-outer: one weight set feeds 8 back-to-back MMs
                for s in range(NS):
                    kw = s % KW
                    kpart = 128 if s < KW else 64
                    for t in range(NT):
                        if s < KW:
                            rhs = Y[:, t * RT : t * RT + RT, kw : kw + W]
                        else:
                            rhs = Y[0:64, t * RT + 2 : t * RT + 2 + RT, kw : kw + W]
                        nc.tensor.matmul(
                            pss[t][:], w16[0:kpart, s, :], rhs,
                            start=(s == 0), stop=(s == NS - 1),
                        )

                for tp in range(NT // 2):
                    t0, t1 = 2 * tp, 2 * tp + 1
                    osb = osbp.tile([O, 2 * RT * W], f32, name="osb")
                    nc.scalar.activation(
                        osb[:, 0 : RT * W], pss[t0][:], Act.Identity,
                        bias=b_sb[:, 0:1], scale=1.0,
                    )
                    nc.vector.tensor_scalar(
                        osb[:, RT * W : 2 * RT * W], pss[t1][:],
                        b_sb[:, 0:1], None, Alu.add,
                    )
                    nc.scalar.dma_start(
                        o_d[:, i, 2 * tp * RT : 2 * tp * RT + 2 * RT, :],
                        osb[:].rearrange("o (r w) -> o r w", r=2 * RT),
                    )
    nc.compile()
    return nc


# ---------------------------------------------------------------- entry


def _prep(inputs):
    x = np.ascontiguousarray(np.asarray(inputs["x"], dtype=np.float32))
    weights = np.ascontiguousarray(np.asarray(inputs["weights"], dtype=np.float32))
    bias = np.ascontiguousarray(np.asarray(inputs["bias"], dtype=np.float32))
    positions = np.ascontiguousarray(
        np.asarray(inputs["positions"], dtype=np.float32)
    )
    return x, weights, bias, positions


def _fast_path_ok(x, positions):
    expect = np.linspace(-1.0, 1.0, P, dtype=np.float32)
    return (
        x.shape == (B, C, H, W)
        and positions.shape == (P,)
        and np.array_equal(positions, expect)
        and float(x.min()) >= 0.0
        and float(x.max()) <= 1.0
    )


def kernel(**inputs):
    x, weights, bias, positions = _prep(inputs)
    if not _fast_path_ok(x, positions):
        return _reference_np(x, weights, bias, positions)

    wstat, bias_eff, ident_any = _host_weights(weights, bias)
    if ident_any:
        # identity-shortcut weights present: needs the raw-v plane; use the
        # exact fallback rather than a rarely-exercised device path
        return _reference_np(x, weights, bias, positions)

    from concourse.bass_utils import run_bass_kernel_spmd

    nc = _build_nc()
    bias2d = np.ascontiguousarray(bias_eff.reshape(O, 1))
    in_maps = [
        {"x": np.ascontiguousarray(x[i * IPC : (i + 1) * IPC]),
         "wstat": wstat, "bias": bias2d}
        for i in range(NCORES)
    ]
    res = run_bass_kernel_spmd(nc, in_maps, core_ids=list(range(NCORES)))
    out = np.concatenate(
        [res.results[i]["out"].transpose(1, 0, 2, 3) for i in range(NCORES)],
        axis=0,
    )
    return np.ascontiguousarray(out)


# ------------------------------------------------------------ dev utilities


def _run_sim(inputs):
    """CoreSim single-core run (images 0..IPC-1) for correctness debugging."""
    from concourse.bass_interp import CoreSim

    x, weights, bias, positions = _prep(inputs)
    assert _fast_path_ok(x, positions)
    wstat, bias_eff, ident_any = _host_weights(weights, bias)
    assert not ident_any
    nc = _build_nc()
    sim = CoreSim(nc)
    sim.tensor("x")[:] = x[:IPC]
    sim.tensor("wstat")[:] = wstat
    sim.tensor("bias")[:] = bias_eff.reshape(O, 1)
    sim.simulate()
    return np.array(sim.tensor("out")).transpose(1, 0, 2, 3)


# revision 13
# speedup vs baseline: 2.2235x; 1.0127x over previous
"""Trainium2 Bass kernel for CustomPositionsPiecewiseConv2d.

Math: for knots positions=[-1,-.5,0,.5,1] and x in [0,1], only the last two
spline intervals are active.  With g2 = 2*min(v,0.5) and c4 = relu(2v-1)
(bf16 rounding absorbs the isclose(v,1) mask), the per-tap contribution is
    contrib = W3 + (1-g2)*(W2-W3) + c4*(W4-W3)
            = W2 + g2*(W3-W2) + c4*(W4-W3)
so  out = G2 (x) (W3-W2) + C4 (x) (W4-W3) + bias + sum_ck W2.

GEMM packing: contraction over (2 planes x 32 ch x 9 taps) = 576 lanes.
Per image one Y buffer [128, 66, 66] holds [g2; c4; g2 row+1; c4 row+1];
a K=128 matmul then contracts taps (0,kw) and (1,kw) simultaneously
(kw = read-time column offset), and the kh=2 row uses K=64 matmuls on the
lower half: 6 matmuls per output tile instead of 9 (or 18 split).  All
plane->Y copies are full-width row windows (contiguous DMA descriptors).

Output is written [O, IPC, H, W] (per-partition-contiguous 4KB segments)
and transposed on host.  Sharding: data-parallel, 2 images per core.
"""

import numpy as np

B, C, H, W = 16, 32, 64, 64
O, P, KH, KW = 128, 5, 3, 3
NCORES = 8
IPC = B // NCORES            # images per core
HP, WP = H + 2, W + 2        # padded plane (pad=1)
RT = 8                       # output rows per L-tile
NT = H // RT                 # L-tiles per image
K2 = KH * KW
NS = 6                       # stationary steps: 3 pair (K=128) + 3 single (K=64)
ATOL = 1e-5
RTOL = 1e-5

# x row chunks for the load->phi->gather->matmul pipeline
RC = [(0, 33), (33, 64)]


# ---------------------------------------------------------------- host math


def _isclose_np(a, b):
    return np.abs(a - b) <= np.float32(ATOL) + np.float32(RTOL) * np.abs(b)


def _reference_np(x, weights, bias, positions):
    """Direct numpy port of the reference (fallback path)."""
    EPS = 1e-6
    Bn, Cn, Hn, Wn = x.shape
    On, _, Pn, KHn, KWn = weights.shape
    xp = np.pad(x, ((0, 0), (0, 0), (1, 1), (1, 1)))
    cols = [
        xp[:, :, i : i + Hn, j : j + Wn] for i in range(KHn) for j in range(KWn)
    ]
    pat = np.stack(cols, axis=2)
    v = pat.reshape(Bn, Cn, KHn * KWn, Hn * Wn).astype(np.float32)

    left, right = positions[:-1], positions[1:]
    denom = right - left
    denom = np.where(denom == 0, np.float32(EPS), denom)
    varc = (1.0 / denom).astype(np.float32)
    const = (-left * varc).astype(np.float32)

    m_first = _isclose_np(v, positions[0])
    m_last = _isclose_np(v, positions[-1])
    in_range = (~(m_first | m_last)) & (v >= positions[0]) & (v <= positions[-1])

    coeff = np.zeros(v.shape + (Pn,), np.float32)
    coeff[..., 0] += m_first.astype(np.float32)
    coeff[..., Pn - 1] += m_last.astype(np.float32)
    for p in range(Pn - 1):
        m = (in_range & (v >= positions[p]) & (v < positions[p + 1])).astype(
            np.float32
        )
        t = v * varc[p] + const[p]
        coeff[..., p] += m * (1.0 - t)
        coeff[..., p + 1] += m * t

    Wk = np.transpose(weights, (0, 1, 3, 4, 2)).reshape(On, Cn, KHn * KWn, Pn)
    ident = np.all(np.abs(Wk - 1.0) <= np.float32(ATOL + RTOL), axis=-1)
    Wk_eff = np.where(ident[..., None], np.float32(0.0), Wk)

    out = np.einsum("bcklp,ockp->bol", coeff, Wk_eff, optimize=True)
    out = out + np.einsum(
        "bckl,ock->bol", v, ident.astype(np.float32), optimize=True
    )
    out = out + bias[None, :, None]
    return out.reshape(Bn, On, Hn, Wn).astype(np.float32)


def _host_weights(weights, bias):
    """Fold the spline into two folded-weight blocks per tap.

    Returns (wstat [128, NS, O] f32, bias_eff [O] f32, ident_any).
    Stationary step s<3 (kw=s): rows = [-P(0,kw); Q(0,kw); -P(1,kw); Q(1,kw)]
    by 32-channel blocks; step s>=3 (kw=s-3): rows = [-P(2,kw); Q(2,kw); 0; 0]
    where P = W2-W3, Q = W4-W3 (so -P multiplies g2 and Q multiplies c4).
    """
    Wk = np.transpose(weights, (0, 1, 3, 4, 2)).reshape(O, C, K2, P)
    ident = np.all(np.abs(Wk - 1.0) <= np.float32(ATOL + RTOL), axis=-1)
    ident_any = bool(ident.any())
    W2 = Wk[:, :, :, 2].astype(np.float64)
    W3 = Wk[:, :, :, 3].astype(np.float64)
    W4 = Wk[:, :, :, 4].astype(np.float64)
    Pm = W2 - W3                      # [O, C, K2]
    Qm = W4 - W3
    # partition layout is channel-interleaved: row 2c+0 = g2 lane of channel
    # c, row 2c+1 = c4 lane (matches the (ch, plane) element order of the
    # plane tile so each Y gather is a single dma_start)
    wstat = np.zeros((128, NS, O), np.float64)
    for kw in range(KW):
        wstat[0:64:2, kw] = -Pm[:, :, 0 * KW + kw].T
        wstat[1:64:2, kw] = Qm[:, :, 0 * KW + kw].T
        wstat[64:128:2, kw] = -Pm[:, :, 1 * KW + kw].T
        wstat[65:128:2, kw] = Qm[:, :, 1 * KW + kw].T
        wstat[0:64:2, KW + kw] = -Pm[:, :, 2 * KW + kw].T
        wstat[1:64:2, KW + kw] = Qm[:, :, 2 * KW + kw].T
    bias_eff = (bias.astype(np.float64) + W2.sum(axis=(1, 2))).astype(np.float32)
    return (
        np.ascontiguousarray(wstat.astype(np.float32)),
        bias_eff,
        ident_any,
    )


# ---------------------------------------------------------------- device IR


def _build_nc():
    import concourse.tile as tile
    from concourse import bacc, mybir

    f32 = mybir.dt.float32
    bf16 = mybir.dt.bfloat16
    Alu = mybir.AluOpType
    Act = mybir.ActivationFunctionType

    nc = bacc.Bacc("TRN2", target_bir_lowering=False, debug=False,
                   num_devices=NCORES)
    x_d = nc.dram_tensor("x", [IPC, C, H, W], f32, kind="ExternalInput").ap()
    w_d = nc.dram_tensor("wstat", [128, NS, O], f32, kind="ExternalInput").ap()
    b_d = nc.dram_tensor("bias", [O, 1], f32, kind="ExternalInput").ap()
    o_d = nc.dram_tensor("out", [O, IPC, H, W], bf16, kind="ExternalOutput").ap()

    with tile.TileContext(nc) as tc:
        with (
            tc.tile_pool(name="const", bufs=1) as constp,
            tc.tile_pool(name="plane", bufs=1) as planep,
            tc.tile_pool(name="ybuf", bufs=2) as ybufp,
            tc.tile_pool(name="psum", bufs=1, space="PSUM") as psump,
            tc.tile_pool(name="osb", bufs=3) as osbp,
        ):
            # ---- x loads first (phi critical path) ----
            XF = planep.tile([IPC * C, H, W], f32)
            for r0, r1 in RC:
                nc.sync.dma_start(
                    XF[:, r0:r1, :],
                    x_d[:, :, r0:r1, :].rearrange("i c h w -> (i c) h w"),
                )

            # pull the ACT table load off the critical path
            tiny = constp.tile([C, 1], f32)
            nc.gpsimd.memset(tiny[:], 0.0)
            nc.scalar.activation(tiny[:], tiny[:], Act.Relu, bias=0.0, scale=1.0)

            # PE warmup: dummy matmuls keep HAM at 8/8 until the real stream
            zb = constp.tile([128, 512], bf16)
            nc.gpsimd.memset(zb[:], 0.0)
            warm_ctr = [0]

            def warm(nmm, rhs=None):
                w = warm_ctr[0]
                warm_ctr[0] += 1
                pw = psump.tile(
                    [O, 512], f32, name=f"ps_warm{w}", tag=f"ps{w % 2}"
                )
                r = zb[:] if rhs is None else rhs
                kp = r.shape[0]
                for j in range(nmm):
                    nc.tensor.matmul(
                        pw[:], zb[0:kp, 0:128], r,
                        start=(j % 8 == 0), stop=(j % 8 == 7 or j == nmm - 1),
                    )

            warm(10)

            # ---- weights ----
            w_sb = constp.tile([128, NS, O], f32)
            nc.sync.dma_start(w_sb[:], w_d[:])
            b_sb = constp.tile([O, 1], f32)
            nc.sync.dma_start(b_sb[:], b_d[:])

            # ---- coefficient planes (both images, 64 partitions) ----
            # TP[:, 0] = g2 plane, TP[:, 1] = c4 plane (one tile so a single
            # dma_start gathers both planes into Y's partition blocks)
            TP = planep.tile([IPC * C, 2, HP, WP], bf16)
            for g in range(2):
                for strip in (
                    TP[:, g, 0, :],
                    TP[:, g, HP - 1, :],
                    TP[:, g, 1 : HP - 1, 0],
                    TP[:, g, 1 : HP - 1, WP - 1],
                ):
                    nc.gpsimd.memset(strip, 0.0)
            negone = constp.tile([IPC * C, 1], f32)
            nc.gpsimd.memset(negone[:], -1.0)
            for r0, r1 in RC:
                nc.vector.tensor_scalar(
                    TP[:, 0, 1 + r0 : 1 + r1, 1 : W + 1], XF[:, r0:r1, :],
                    0.5, 2.0, Alu.min, Alu.mult,
                )
                nc.scalar.activation(
                    TP[:, 1, 1 + r0 : 1 + r1, 1 : W + 1], XF[:, r0:r1, :],
                    Act.Relu, bias=negone[:], scale=2.0,
                )

            w16 = constp.tile([128, NS, O], bf16)
            nc.vector.tensor_copy(w16[:], w_sb[:])

            # bridge PE busy-ness across the phi phase (HAM re-throttles
            # after ~3.4us idle); each batch gated on a later phi artifact
            warm(5, TP[0:64, 0, 0:RT, 0:W])
            warm(5, TP[0:64, 1, 0:RT, 0:W])

            # ---- per-image gather + GEMM ----
            def gather(i, Y, chunks):
                # chunks: disjoint ((lo0, lo1), (hi0, hi1)) plane-row ranges;
                # lower 64 parts hold plane rows r, upper 64 hold rows r+1.
                # One dma_start per block: src [32ch, 2planes, R, WP] ->
                # dst partitions (plane*32 + ch).
                s = slice(i * C, (i + 1) * C)
                for (lo0, lo1), (hi0, hi1) in chunks:
                    nc.sync.dma_start(
                        Y[0:64, lo0:lo1, :], TP[s, :, lo0:lo1, :]
                    )
                    nc.sync.dma_start(
                        Y[64:128, hi0:hi1, :], TP[s, :, hi0 + 1 : hi1 + 1, :]
                    )

            for i in range(IPC):
                Y = ybufp.tile([128, HP, WP], bf16, name="Y", tag="Y")
                if i == 0:
                    # chunked: matmul tiles 0-3 start after the first chunk
                    gather(i, Y, [((0, 34), (0, 33)), ((34, HP), (33, H))])
                else:
                    gather(i, Y, [((0, HP), (0, H))])

                pss = [
                    psump.tile([O, RT * W], f32, name=f"ps{t}", tag=f"ps{t}")
                    for t in range(NT)
                ]
                # stationary-outer: one weight set feeds 8 back-to-back MMs
                for s in range(NS):
                    kw = s % KW
                    kpart = 128 if s < KW else 64
                    for t in range(NT):
                        if s < KW:
                            rhs = Y[:, t * RT : t * RT + RT, kw : kw + W]
                        else:
                            rhs = Y[0:64, t * RT + 2 : t * RT + 2 + RT, kw : kw + W]
                        nc.tensor.matmul(
                            pss[t][:], w16[0:kpart, s, :], rhs,
                            start=(s == 0), stop=(s == NS - 1),
                        )

                for tp in range(NT // 2):
                    t0, t1 = 2 * tp, 2 * tp + 1
                    osb = osbp.tile([O, 2 * RT * W], bf16, name="osb")
                    nc.scalar.activation(
                        osb[:, 0 : RT * W], pss[t0][:], Act.Identity,
                        bias=b_sb[:, 0:1], scale=1.0,
                    )
                    nc.vector.tensor_scalar(
                        osb[:, RT * W : 2 * RT * W], pss[t1][:],
                        b_sb[:, 0:1], None, Alu.add,
                    )
                    nc.scalar.dma_start(
                        o_d[:, i, 2 * tp * RT : 2 * tp * RT + 2 * RT, :],
                        osb[:].rearrange("o (r w) -> o r w", r=2 * RT),
                    )
    nc.compile()
    return nc


# ---------------------------------------------------------------- entry


def _prep(inputs):
    x = np.ascontiguousarray(np.asarray(inputs["x"], dtype=np.float32))
    weights = np.ascontiguousarray(np.asarray(inputs["weights"], dtype=np.float32))
    bias = np.ascontiguousarray(np.asarray(inputs["bias"], dtype=np.float32))
    positions = np.ascontiguousarray(
        np.asarray(inputs["positions"], dtype=np.float32)
    )
    return x, weights, bias, positions


def _fast_path_ok(x, positions):
    expect = np.linspace(-1.0, 1.0, P, dtype=np.float32)
    return (
        x.shape == (B, C, H, W)
        and positions.shape == (P,)
        and np.array_equal(positions, expect)
        and float(x.min()) >= 0.0
        and float(x.max()) <= 1.0
    )


def kernel(**inputs):
    x, weights, bias, positions = _prep(inputs)
    if not _fast_path_ok(x, positions):
        return _reference_np(x, weights, bias, positions)

    wstat, bias_eff, ident_any = _host_weights(weights, bias)
    if ident_any:
        # identity-shortcut weights present: needs the raw-v plane; use the
        # exact fallback rather than a rarely-exercised device path
        return _reference_np(x, weights, bias, positions)

    from concourse.bass_utils import run_bass_kernel_spmd

    nc = _build_nc()
    bias2d = np.ascontiguousarray(bias_eff.reshape(O, 1))
    in_maps = [
        {"x": np.ascontiguousarray(x[i * IPC : (i + 1) * IPC]),
         "wstat": wstat, "bias": bias2d}
        for i in range(NCORES)
    ]
    res = run_bass_kernel_spmd(nc, in_maps, core_ids=list(range(NCORES)))
    out = np.concatenate(
        [
            np.asarray(res.results[i]["out"])
            .astype(np.float32)
            .transpose(1, 0, 2, 3)
            for i in range(NCORES)
        ],
        axis=0,
    )
    return np.ascontiguousarray(out)


# ------------------------------------------------------------ dev utilities


def _run_sim(inputs):
    """CoreSim single-core run (images 0..IPC-1) for correctness debugging."""
    from concourse.bass_interp import CoreSim

    x, weights, bias, positions = _prep(inputs)
    assert _fast_path_ok(x, positions)
    wstat, bias_eff, ident_any = _host_weights(weights, bias)
    assert not ident_any
    nc = _build_nc()
    sim = CoreSim(nc)
    sim.tensor("x")[:] = x[:IPC]
    sim.tensor("wstat")[:] = wstat
    sim.tensor("bias")[:] = bias_eff.reshape(O, 1)
    sim.simulate()
    return (
        np.array(sim.tensor("out")).astype(np.float32).transpose(1, 0, 2, 3)
    )
